# revision 5
# baseline (speedup 1.0000x reference)
"""Cross-attention kernel for 8 Trainium2 NeuronCores (v3).

Problem: nn_CrossAttention (N=2, X=1024, T=4096, D=1024, H=16, hd=64).

Sharding: core c handles batch n = c//4 and head-group hg = c%4
(4 heads = 256 output dims). No cross-core communication.

All-bf16 dataflow (fp8 fails the 2e-2 gate: per-element noise on p or v
shows up at FULL rms in the output because softmax output norm carries
the same sqrt(eff)-cancellation as the noise). v3 vs the 219us baseline:
  - O' (P@V) matmuls are deferred TWO steps behind their scores, so the
    exp has two full steps to complete and the O' never stalls the PE
    (~200ns/step of PE idle recovered).
  - Drain transposes run in bf16 (half the fp32 transpose cost).
  - bv folded into the host epilogue (softmax weights sum to 1); the v
    bias-add becomes a plain PSUM->SBUF copy.
  - 1/5 of exp tiles run on the DVE via an int16 Schraudolph bit-trick
    (bf16 bits = RNE(23.083*s + 16250.5); ~1.9% rms on those tiles,
    ~0.85% overall) to keep ScalarE off the critical path.
"""

import sys
import types

import numpy as np
import ml_dtypes
from contextlib import ExitStack

try:
    import antenv.axon_hooks  # noqa: F401
except ImportError:
    _m = types.ModuleType("antenv.axon_hooks")
    _m.get_axon_ntff_profile_hook = lambda: None
    _m.set_axon_ntff_profile_hook = lambda h: None
    sys.modules["antenv.axon_hooks"] = _m
    try:
        import antenv
        antenv.axon_hooks = _m
    except ImportError:
        pass

import concourse.bacc as bacc
import concourse.tile as tile
import concourse.mybir as mybir
from concourse.bass_utils import run_bass_kernel_spmd
from concourse.masks import make_identity

D, H, HD = 1024, 16, 64
N, X, T = 2, 1024, 4096
NCORES = 8
CH = 4            # heads per core
CW = CH * HD      # 256 output cols per core
KT = D // 128     # 8 d-tiles
TT = T // 128     # 32 t-tiles
XTILES = X // 128
BF16 = mybir.dt.bfloat16
F32 = mybir.dt.float32
I16 = mybir.dt.int16
EXP = mybir.ActivationFunctionType.Exp
ALU = mybir.AluOpType

# ScalarE path: p = exp(s/8).  DVE path: bf16 bits i = a*(s) + b with
# a = (128/ln2)*0.125, b = 127*128 - 0.0431*128 (Schraudolph-centered).
SCL = 0.125
A_ = (128.0 / np.log(2.0)) * 0.125
B_ = 16256.0 - 5.52

_CACHE = {}


def _build_program():
    nc = bacc.Bacc("TRN2", target_bir_lowering=False, debug=False,
                   num_devices=NCORES)

    xt_d = nc.dram_tensor("xt", (2, 128, KT, 512), BF16, kind="ExternalInput")
    ctxt_d = nc.dram_tensor("ctxt", (8, 128, KT, 512), BF16,
                            kind="ExternalInput")
    wqt_d = nc.dram_tensor("wqt", (128, KT, CW), BF16, kind="ExternalInput")
    wkt_d = nc.dram_tensor("wkt", (128, KT, CW), BF16, kind="ExternalInput")
    wvt_d = nc.dram_tensor("wvt", (128, KT, CW), BF16, kind="ExternalInput")
    bq_d = nc.dram_tensor("bq", (128, 2), F32, kind="ExternalInput")
    bk_d = nc.dram_tensor("bk", (128, 2), F32, kind="ExternalInput")
    out_d = nc.dram_tensor("out", (X, CW), F32, kind="ExternalOutput")

    gctr = iter(range(4 * TT))   # exp-engine pattern counter

    with tile.TileContext(nc) as tc, ExitStack() as ctx:
        consts = ctx.enter_context(tc.tile_pool(name="consts", bufs=1))
        pt_pool = ctx.enter_context(tc.tile_pool(name="pt", bufs=4))
        osb_pool = ctx.enter_context(tc.tile_pool(name="osb", bufs=2))
        rc_pool = ctx.enter_context(tc.tile_pool(name="rc", bufs=2))
        mp = ctx.enter_context(tc.tile_pool(name="mp", bufs=4, space="PSUM"))
        st_pool = ctx.enter_context(
            tc.tile_pool(name="st", bufs=2, space="PSUM"))

        # ---- resident SBUF tensors ----
        wq_sb = consts.tile([128, KT, CW], BF16)
        wk_sb = consts.tile([128, KT, CW], BF16)
        wv_sb = consts.tile([128, KT, CW], BF16)
        xt_sb = consts.tile([128, KT, X], BF16)
        ctx_sb = consts.tile([128, KT, T], BF16)
        qt_sb = consts.tile([128, 2, X], BF16)
        kt_sb = consts.tile([128, 2, T], BF16)
        vp_sb = consts.tile([128, TT, CH * (HD + 1)], BF16)
        out_sb = consts.tile([128, XTILES, CW], F32)
        bq_sb = consts.tile([128, 2], F32)
        bk_sb = consts.tile([128, 2], F32)
        ident = consts.tile([128, 128], BF16)

        vp_h = vp_sb[:].rearrange("p t (h c) -> p t h c", c=HD + 1)

        # ---- PE warm-up: dummy matmuls while input DMAs land ----
        dumin = consts.tile([128, 512], BF16)
        nc.gpsimd.memset(dumin[:], 0.0)
        dps = mp.tile([128, 512], F32, tag="mp", name="dps")
        for i in range(10):
            nc.tensor.matmul(dps[:], dumin[:, 0:128], dumin[:],
                             start=(i == 0), stop=(i == 9))

        # ---- input DMAs (ordered so compute can start early) ----
        def ctx_dma(c):
            nc.sync.dma_start(ctx_sb[:, :, 512 * c:512 * (c + 1)],
                              ctxt_d.ap()[c])

        nc.sync.dma_start(wq_sb[:], wqt_d.ap())
        nc.sync.dma_start(xt_sb[:, :, 0:512], xt_d.ap()[0])
        nc.sync.dma_start(bq_sb[:], bq_d.ap())
        nc.sync.dma_start(wk_sb[:], wkt_d.ap())
        ctx_dma(0)
        nc.sync.dma_start(bk_sb[:], bk_d.ap())
        nc.sync.dma_start(wv_sb[:], wvt_d.ap())
        ctx_dma(1)
        nc.sync.dma_start(xt_sb[:, :, 512:1024], xt_d.ap()[1])
        for c in range(2, 8):
            ctx_dma(c)
        make_identity(nc, ident[:])
        nc.gpsimd.memset(vp_h[:, :, :, HD:HD + 1], 1.0)

        # ---- projections ----
        def qt_proj(ct, xc):
            ps = mp.tile([128, 512], F32, tag="mp", name=f"qps{ct}{xc}")
            for dt in range(KT):
                nc.tensor.matmul(
                    ps[:],
                    wq_sb[:, dt, 128 * ct:128 * (ct + 1)],
                    xt_sb[:, dt, 512 * xc:512 * (xc + 1)],
                    start=(dt == 0), stop=(dt == KT - 1))
            nc.vector.tensor_scalar_add(
                qt_sb[:, ct, 512 * xc:512 * (xc + 1)], ps[:],
                bq_sb[:, ct:ct + 1])

        def kt_chunk(ct, c):
            ps = mp.tile([128, 512], F32, tag="mp", name=f"kps{ct}_{c}")
            for dt in range(KT):
                nc.tensor.matmul(
                    ps[:],
                    wk_sb[:, dt, 128 * ct:128 * (ct + 1)],
                    ctx_sb[:, dt, 512 * c:512 * (c + 1)],
                    start=(dt == 0), stop=(dt == KT - 1))
            nc.vector.tensor_scalar_add(
                kt_sb[:, ct, 512 * c:512 * (c + 1)], ps[:],
                bk_sb[:, ct:ct + 1])

        def v_tile(tt):
            ps = mp.tile([128, 512], F32, tag="mp", name=f"vps{tt}")
            for dt in range(KT):
                nc.tensor.matmul(
                    ps[:, 0:CW],
                    ctx_sb[:, dt, 128 * tt:128 * (tt + 1)],
                    wv_sb[:, dt, :],
                    start=(dt == 0), stop=(dt == KT - 1))
            nc.vector.tensor_copy(
                vp_h[:, tt, :, 0:HD],
                ps[:, 0:CW].rearrange("p (h c) -> p h c", c=HD))

        qt_proj(0, 0)
        kt_chunk(0, 0)

        # ---- attention ----
        oacc = {}
        ptq = {}

        def attn_start(hp, xc):
            oacc[(hp, xc)] = [
                mp.tile([65, 512], F32, tag="mp", name=f"oacc{hp}{xc}{h2}")
                for h2 in range(2)]

        def score_step(hp, xc, tt):
            st = st_pool.tile([128, 1024], F32, tag="st",
                              name=f"st{hp}{xc}{tt}")
            for h2 in range(2):
                nc.tensor.matmul(
                    st[:, 512 * h2:512 * (h2 + 1)],
                    kt_sb[64 * h2:64 * (h2 + 1), hp,
                          128 * tt:128 * (tt + 1)],
                    qt_sb[64 * h2:64 * (h2 + 1), hp,
                          512 * xc:512 * (xc + 1)],
                    start=True, stop=True)
            pt = pt_pool.tile([128, 1024], BF16, tag="pt",
                              name=f"pt{hp}{xc}{tt}")
            if next(gctr) % 5 == 4:
                nc.vector.tensor_scalar(pt[:].bitcast(I16), st[:], A_, B_,
                                        ALU.mult, ALU.add)
            else:
                nc.scalar.activation(pt[:], st[:], EXP, scale=SCL)
            ptq[(hp, xc, tt)] = pt

        def ov_step(hp, xc, tt):
            pt = ptq.pop((hp, xc, tt))
            for h2 in range(2):
                h = 2 * hp + h2
                nc.tensor.matmul(
                    oacc[(hp, xc)][h2][:],
                    vp_sb[:, tt, 65 * h:65 * (h + 1)],
                    pt[:, 512 * h2:512 * (h2 + 1)],
                    start=(tt == 0), stop=(tt == TT - 1))

        def attn_drain(hp, xc, out_ap=None):
            ots = []
            for h2 in range(2):
                ot = osb_pool.tile([65, 512], BF16, tag="osb",
                                   name=f"ot{hp}{xc}{h2}")
                nc.vector.tensor_copy(ot[:], oacc[(hp, xc)][h2][:])
                ots.append(ot)
            for s in range(4):
                for h2 in range(2):
                    h = 2 * hp + h2
                    tp = mp.tile([128, 65], BF16, tag="mp",
                                 name=f"tp{hp}{xc}{h2}{s}")
                    nc.tensor.transpose(
                        tp[:], ots[h2][:, 128 * s:128 * (s + 1)],
                        ident[0:65, 0:65])
                    rc = rc_pool.tile([128, 1], F32, tag="rc",
                                      name=f"rc{hp}{xc}{h2}{s}")
                    nc.vector.reciprocal(rc[:], tp[:, 64:65])
                    nc.vector.tensor_scalar_mul(
                        out_sb[:, 4 * xc + s, 64 * h:64 * (h + 1)],
                        tp[:, 0:64], rc[:])
                if out_ap is not None:
                    nc.sync.dma_start(out_ap[:, 4 * xc + s:4 * xc + s + 1],
                                      out_sb[:, 4 * xc + s:4 * xc + s + 1])
            del oacc[(hp, xc)]

        out_ap = out_d.ap().rearrange("(xt p) c -> p xt c", p=128)

        def run_stream(hp, xc, fillers, tail_fillers=()):
            """Scores run two steps ahead of the O' accumulation, so exp
            latency is fully hidden.  fillers[i] runs between score(i) and
            the deferred ov-step."""
            attn_start(hp, xc)
            score_step(hp, xc, 0)
            for f in fillers.get(-1, ()):
                f()
            score_step(hp, xc, 1)
            for tt in range(2, TT):
                score_step(hp, xc, tt)
                for f in fillers.get(tt, ()):
                    f()
                ov_step(hp, xc, tt - 2)
            for f in tail_fillers:
                f()
            ov_step(hp, xc, TT - 2)
            ov_step(hp, xc, TT - 1)

        # stream (0,0): kt ct0 chunks + all v tiles ride inside
        f = {-1: [lambda: v_tile(0), lambda: v_tile(1)]}
        for tt in range(2, TT):
            fl = [lambda t=tt: v_tile(t)]
            if tt % 4 == 2 and tt // 4 + 1 < KT:
                fl.append(lambda c=tt // 4 + 1: kt_chunk(0, c))
            if tt == 29:
                fl.append(lambda: qt_proj(0, 1))
            f[tt] = fl
        run_stream(0, 0, f, tail_fillers=(lambda: kt_chunk(1, 0),))

        # stream (0,1): drain(0,0), kt ct1 chunks 1-3, qt(1,0)
        f = {7: [lambda: attn_drain(0, 0)],
             11: [lambda: kt_chunk(1, 1)],
             17: [lambda: kt_chunk(1, 2)],
             23: [lambda: kt_chunk(1, 3)],
             29: [lambda: qt_proj(1, 0)]}
        run_stream(0, 1, f)

        # stream (1,0): drain(0,1), kt ct1 chunks 4-7, qt(1,1)
        f = {7: [lambda: attn_drain(0, 1)],
             9: [lambda: kt_chunk(1, 4)],
             11: [lambda: kt_chunk(1, 5)],
             13: [lambda: kt_chunk(1, 6)],
             15: [lambda: kt_chunk(1, 7)],
             21: [lambda: qt_proj(1, 1)]}
        run_stream(1, 0, f)

        # stream (1,1)
        f = {7: [lambda: attn_drain(1, 0, out_ap)]}
        run_stream(1, 1, f)
        attn_drain(1, 1, out_ap)

    nc.compile()
    return nc


def get_program():
    if "nc" not in _CACHE:
        _CACHE["nc"] = _build_program()
    return _CACHE["nc"]


def _swizzle(at, inner):
    """(D, M) d-major -> (M//inner, 128, KT, inner): chunked, partition-
    contiguous rows so each DMA descriptor is a long linear run."""
    dd, m = at.shape
    return np.ascontiguousarray(
        at.reshape(KT, 128, m // inner, inner).transpose(2, 1, 0, 3))


def _shard_inputs(previous_output, context, Wq, bq, Wk, bk, Wv, bv):
    bf = ml_dtypes.bfloat16
    xt = [_swizzle(previous_output[n].T.astype(bf), 512) for n in range(N)]
    ctxt = [_swizzle(context[n].T.astype(bf), 512) for n in range(N)]
    in_maps = []
    for c in range(NCORES):
        n, hg = c // CH, c % CH
        sl = slice(CW * hg, CW * (hg + 1))
        in_maps.append({
            "xt": xt[n],
            "ctxt": ctxt[n],
            "wqt": _swizzle(Wq[sl].T.astype(bf), CW)[0],
            "wkt": _swizzle(Wk[sl].T.astype(bf), CW)[0],
            "wvt": _swizzle(Wv[sl].T.astype(bf), CW)[0],
            "bq": np.ascontiguousarray(
                bq[sl].reshape(2, 128).T).astype(np.float32),
            "bk": np.ascontiguousarray(
                bk[sl].reshape(2, 128).T).astype(np.float32),
        })
    return in_maps


LAST_RESULTS = None


def kernel(previous_output, context, Wq, bq, Wk, bk, Wv, bv):
    global LAST_RESULTS
    previous_output = np.asarray(previous_output, dtype=np.float32)
    context = np.asarray(context, dtype=np.float32)
    Wq = np.asarray(Wq, dtype=np.float32)
    Wk = np.asarray(Wk, dtype=np.float32)
    Wv = np.asarray(Wv, dtype=np.float32)
    bq = np.asarray(bq, dtype=np.float32)
    bk = np.asarray(bk, dtype=np.float32)
    bv = np.asarray(bv, dtype=np.float32)

    nc = get_program()
    in_maps = _shard_inputs(previous_output, context, Wq, bq, Wk, bk, Wv, bv)
    res = run_bass_kernel_spmd(nc, in_maps, core_ids=list(range(NCORES)))
    LAST_RESULTS = res

    out = np.empty((N, X, D), dtype=np.float32)
    for c in range(NCORES):
        n, hg = c // CH, c % CH
        sl = slice(CW * hg, CW * (hg + 1))
        out[n, :, sl] = res.results[c]["out"] + bv[sl]
    return out
